# revision 20
# baseline (speedup 1.0000x reference)
"""BigBird block-sparse attention for Trainium2, 8-core SPMD.

Sharding: head-parallel. Each core owns 2 of the 16 heads (both batches).
  - q/k/v projections computed only for the core's 128 feature slice
    (full hidden_states replicated, weights sliced column-wise).
  - attention fully local per (batch, head).
  - out_proj tensor-parallel on the head (contraction) dim: each core
    emits a full-shape partial; the host sums the 8 partials (cheaper
    than a 16MB on-device all-reduce) and adds the output bias.

On-device layout choices:
  - activations feature-major (features on partitions, tokens on free dim)
  - scores computed transposed: S_T[key, query] = k_j^T q, so that
    * AV is a natural matmul (contraction = keys = partitions),
    * the softmax denominator Z falls out of a ones-column appended to V^T,
    * normalization folds into the PSUM->SBUF context copy as a
      partition-broadcast multiply by 1/Z.
  - softmax skips max-subtraction (scores are O(1) after the 1/8 scale;
    exp cannot overflow fp32 for this distribution; softmax is shift
    invariant so the reference is matched).
  - BigBird mask is data independent and block-constant (64x64): it is
    evaluated at trace time into run-lists of attending query blocks per
    128-wide key tile.  No mask tensors on device at all.
"""

import numpy as np
import ml_dtypes
from contextlib import ExitStack

# ----- problem constants (hardcoded per contract) --------------------------
EMBED_DIM = 1024
NUM_HEADS = 16
HEAD_DIM = 64           # d per head
WINDOW = 3
N_RAND = 3
BLOCK = 64
BATCH = 2
SEQ = 2048
NB = SEQ // BLOCK       # 32 key/query blocks per sequence
N_CORES = 8
HPC = NUM_HEADS // N_CORES      # heads per core = 2
FPC = HPC * HEAD_DIM            # feature slice per core = 128
T = BATCH * SEQ                 # 4096 tokens
NKT = NB // 2                   # 16 key tiles of 128 keys per (b,h)
SCALE = HEAD_DIM ** -0.5

BF16 = ml_dtypes.bfloat16

# score-chunk window width in psum columns (2 PSUM banks)
CHUNK_W = 1024
PSUM_BANK = 512  # fp32 elements per bank


def _block_attend() -> np.ndarray:
    """attend[r, kb]: query block r attends key block kb.

    Block-granular replica of the reference _bigbird_mask (the mask is
    block-constant: global first block rows/cols, +-WINDOW band, and
    N_RAND random blocks per row drawn with RandomState(0))."""
    att = np.zeros((NB, NB), dtype=bool)
    att[0, :] = True
    att[:, 0] = True
    blk = np.arange(NB)
    att |= np.abs(blk[:, None] - blk[None, :]) <= WINDOW
    rng = np.random.RandomState(0)
    for b in range(1, NB):
        avail = [x for x in range(1, NB) if abs(x - b) > WINDOW]
        if avail:
            sel = rng.choice(avail, size=min(N_RAND, len(avail)), replace=False)
            att[b, sel] = True
    return att


def _runs_of(mask_1d: np.ndarray):
    """[(r0, nblocks)] maximal runs of consecutive True entries."""
    runs = []
    for r in np.flatnonzero(mask_1d):
        if runs and runs[-1][0] + runs[-1][1] == r:
            runs[-1][1] += 1
        else:
            runs.append([int(r), 1])
    return [(r0, n) for r0, n in runs]


def build_schedule():
    """Per key-tile j, pack score pieces into <=CHUNK_W-wide psum windows.

    Returns list of chunks; each chunk is a dict:
      j      : key tile index (keys j*128 .. j*128+128)
      W      : used width in psum columns
      segs   : [(side, r0, nblk, off)]  real score/AV pieces
               side: 2=full tile (128 keys), 0=low half (kb 2j), 1=high half
               r0   : first query block, nblk consecutive blocks
               off  : chunk-local psum column offset (64*... aligned)
      fillers: [(side, off, w)] dummy score MMs so exp never reads
               unwritten psum (output discarded; AV never touches them)
    """
    att = _block_attend()
    chunks = []
    for j in range(NKT):
        kb0, kb1 = 2 * j, 2 * j + 1
        a0, a1 = att[:, kb0], att[:, kb1]
        segs_all = []   # (side, r0, nblk) in absolute layout order
        # full pieces first, then the two half-piece streams sharing columns
        full_runs = _runs_of(a0 & a1)
        h0_runs = _runs_of(a0 & ~a1)
        h1_runs = _runs_of(a1 & ~a0)

        # absolute column layout: full region, then overlap region for halves
        layout = []  # (side, r0, nblk, abs_off)
        off = 0
        for r0, n in full_runs:
            layout.append((2, r0, n, off))
            off += 64 * n
        half_base = off
        off0 = off1 = half_base
        for r0, n in h0_runs:
            layout.append((0, r0, n, off0))
            off0 += 64 * n
        for r0, n in h1_runs:
            layout.append((1, r0, n, off1))
            off1 += 64 * n
        W_total = max(off0, off1)

        # slice the absolute layout into CHUNK_W windows
        n_windows = max(1, -(-W_total // CHUNK_W))
        for w_i in range(n_windows):
            lo, hi = w_i * CHUNK_W, min((w_i + 1) * CHUNK_W, W_total)
            segs = []
            cover = [np.zeros(hi - lo, bool), np.zeros(hi - lo, bool)]
            for side, r0, n, aoff in layout:
                s, e = aoff, aoff + 64 * n
                cs, ce = max(s, lo), min(e, hi)
                if cs >= ce:
                    continue
                # clip to window; r advances with columns (64 per block)
                r_lo = r0 + (cs - s) // 64
                nblk = (ce - cs) // 64
                segs.append((side, r_lo, nblk, cs - lo))
                for sd in ((0, 1) if side == 2 else (side,)):
                    cover[sd][cs - lo:ce - lo] = True
            if not segs:
                continue
            used = max(o + 64 * n for (_s, _r, n, o) in segs)
            fillers = []
            for sd in (0, 1):
                m = ~cover[sd][:used]
                i = 0
                while i < used:
                    if m[i]:
                        k = i
                        while k < used and m[k]:
                            k += 1
                        fillers.append((sd, i, k - i))
                        i = k
                    else:
                        i += 1
            chunks.append(dict(j=j, W=used, segs=segs, fillers=fillers))
    return chunks


def _bank_split(off, w, bank=PSUM_BANK):
    """split [off, off+w) at bank boundaries -> [(off, w), ...]"""
    out = []
    while w > 0:
        room = bank - (off % bank)
        take = min(room, w)
        out.append((off, take))
        off += take
        w -= take
    return out


# ---------------------------------------------------------------------------
# numpy golden of the exact on-device algorithm (fp32, validates schedule)
# ---------------------------------------------------------------------------
def numpy_golden(hidden_states, wq, bq, wk, bk, wv, bv, wo, bo):
    hs = np.asarray(hidden_states, np.float32).reshape(T, EMBED_DIM)
    chunks = build_schedule()
    out = np.zeros((T, EMBED_DIM), np.float32)
    for c in range(N_CORES):
        f = slice(FPC * c, FPC * (c + 1))
        q = hs @ np.asarray(wq, np.float32)[f, :].T  # (T, 128)
        k = hs @ np.asarray(wk, np.float32)[f, :].T
        v = hs @ np.asarray(wv, np.float32)[f, :].T
        partial = np.zeros((EMBED_DIM, T), np.float32)
        ctx_all = np.zeros((FPC, T), np.float32)
        for b in range(BATCH):
            for hl in range(HPC):
                d = slice(64 * hl, 64 * hl + 64)
                tok = slice(b * SEQ, (b + 1) * SEQ)
                qb = q[tok, d]   # (2048, 64)
                kb = k[tok, d]
                vb = v[tok, d]
                v_aug = np.concatenate([vb, np.ones((SEQ, 1), np.float32)], 1)
                ctx = np.zeros((65, SEQ), np.float32)
                for ch in chunks:
                    j = ch["j"]
                    E = np.zeros((128, ch["W"]), np.float32)
                    for side, r0, nblk, off in ch["segs"]:
                        kk = (slice(j * 128, j * 128 + 128) if side == 2 else
                              slice(j * 128 + 64 * side, j * 128 + 64 * side + 64))
                        qq = slice(64 * r0, 64 * (r0 + nblk))
                        s = kb[kk, :] @ qb[qq, :].T  # (keys, queries)
                        E[0 if side in (0, 2) else 64:][:s.shape[0], off:off + 64 * nblk] = \
                            np.exp(SCALE * s)
                    for side, r0, nblk, off in ch["segs"]:
                        kk = (slice(j * 128, j * 128 + 128) if side == 2 else
                              slice(j * 128 + 64 * side, j * 128 + 64 * side + 64))
                        rows = slice(0, 128) if side == 2 else \
                            slice(64 * side, 64 * side + 64)
                        qq = slice(64 * r0, 64 * (r0 + nblk))
                        ctx[:, qq] += v_aug[kk, :].T @ E[rows, off:off + 64 * nblk]
                ctx_n = ctx[:64, :] / ctx[64:65, :]
                ctx_all[d, tok] = ctx_n
        partial = np.asarray(wo, np.float32)[:, f] @ ctx_all  # (1024, T)
        out += partial.T
    out = out + np.asarray(bo, np.float32)
    return out.reshape(BATCH, SEQ, EMBED_DIM)


# ---------------------------------------------------------------------------
# Bass/Tile kernel (one core's program; SPMD across 8 cores)
# ---------------------------------------------------------------------------
def _trace_core_program():
    import concourse.bass as bass
    import concourse.mybir as mybir
    import concourse.tile as tile
    from concourse import bacc

    dt = mybir.dt
    chunks = build_schedule()

    nc = bacc.Bacc(None, target_bir_lowering=False)
    with tile.TileContext(nc) as tc:
        with ExitStack() as top:
            dram = top.enter_context(tc.tile_pool(name="dram", bufs=1, space="DRAM"))
            hT_d = dram.tile([EMBED_DIM, T], dt.bfloat16, kind="ExternalInput",
                             name="hT", uniquify=False)
            wqkT_d = dram.tile([EMBED_DIM, 2 * FPC], dt.bfloat16,
                               kind="ExternalInput", name="wqkT", uniquify=False)
            wvT_d = dram.tile([EMBED_DIM, FPC], dt.bfloat16,
                              kind="ExternalInput", name="wvT", uniquify=False)
            woT_d = dram.tile([FPC, EMBED_DIM], dt.bfloat16,
                              kind="ExternalInput", name="woT", uniquify=False)
            ident_d = dram.tile([128, 64], dt.bfloat16,
                                kind="ExternalInput", name="ident",
                                uniquify=False)
            out_d = dram.tile([EMBED_DIM, T], dt.bfloat16,
                              kind="ExternalOutput", name="out", uniquify=False)

            # ---- persistent SBUF tensors -----------------------------------
            persist = top.enter_context(tc.tile_pool(name="persist", bufs=1))
            hT = persist.tile([128, 8, T], dt.bfloat16, name="hT_sb")
            wqk = persist.tile([128, 8, 2 * FPC], dt.bfloat16, name="wqk_sb")
            wvT = persist.tile([128, 8, FPC], dt.bfloat16, name="wv_sb")
            woT = persist.tile([128, EMBED_DIM], dt.bfloat16, name="wo_sb")
            # q/k/v head-major on 64 partitions (base-0 only: matmuls with
            # base-partition-64 contraction operands hit a codegen/HW bug)
            q_sb = persist.tile([64, HPC * T], dt.bfloat16, name="q_sb")
            k_sb = persist.tile([64, HPC * T], dt.bfloat16, name="k_sb")
            vfm = persist.tile([64, HPC * T], dt.bfloat16, name="vfm_sb")
            # per (b, hl): zero-padded v^T slots, one per 64-key block m:
            # rows (m%2)*64..+64 hold [v | 1], the other 64 rows are zero,
            # so every AV matmul is K=128 at base partition 0.
            vaug = persist.tile([128, BATCH * HPC, NB * 65], dt.bfloat16,
                                name="vaug_sb")
            ctx_all = persist.tile([128, T], dt.bfloat16, name="ctx_sb")
            ident = persist.tile([128, 64], dt.bfloat16, name="ident_sb")

            # input DMAs (feature tiles e: partitions are e*128..)
            nc.sync.dma_start(out=hT[:], in_=hT_d.rearrange(
                "(e p) t -> p e t", p=128))
            nc.sync.dma_start(out=wqk[:], in_=wqkT_d.rearrange(
                "(e p) f -> p e f", p=128))
            nc.sync.dma_start(out=wvT[:], in_=wvT_d.rearrange(
                "(e p) f -> p e f", p=128))
            nc.sync.dma_start(out=woT[:], in_=woT_d[:])
            nc.sync.dma_start(out=ident[:], in_=ident_d[:])

            # ---- phase 1: q/k/v projections (feature-major) ----------------
            NCHUNK = T // 512
            with tc.tile_pool(name="proj_ps", bufs=2, space="PSUM") as pps:
                for n in range(NCHUNK):
                    tsl = slice(512 * n, 512 * (n + 1))
                    for which, (wsl, dst) in enumerate(
                            [(slice(0, 128), q_sb), (slice(128, 256), k_sb),
                             (None, vfm)]):
                        ps = pps.tile([128, 512], dt.float32, tag="proj")
                        for e in range(8):
                            lhsT = (wqk[:, e, wsl] if wsl is not None
                                    else wvT[:, e, :])
                            nc.tensor.matmul(ps[:], lhsT, hT[:, e, tsl],
                                             start=(e == 0), stop=(e == 7))
                        # copy+cast psum -> sbuf per head half, alternating
                        # engines (dst is head-major on 64 partitions)
                        for hl in range(HPC):
                            eng = (2 * which + hl + n) % 2
                            src = ps[64 * hl:64 * hl + 64, :]
                            d2 = dst[:, hl * T + 512 * n: hl * T + 512 * n + 512]
                            if eng:
                                nc.scalar.copy(d2, src)
                            else:
                                nc.vector.tensor_copy(d2, src)

            # ---- phase 2: v -> token-major, zero-padded per-block slots ----
            with tc.tile_pool(name="vt_ps", bufs=4, space="PSUM") as vtp:
                # zero the pad halves + set the ones columns (disjoint from
                # the data ranges the copies below write)
                for p in range(BATCH * HPC):
                    slots = vaug[:, p, :].rearrange("p (m c) -> p m c", c=65)
                    nc.vector.memset(slots[0:64, 1::2, :], 0.0)
                    nc.vector.memset(slots[64:128, 0::2, :], 0.0)
                    nc.vector.memset(slots[0:64, 0::2, 64:65], 1.0)
                    nc.vector.memset(slots[64:128, 1::2, 64:65], 1.0)
                for b in range(BATCH):
                    for hl in range(HPC):
                        p = b * HPC + hl
                        for jj in range(NKT):
                            vt = vtp.tile([128, 64], dt.bfloat16, tag="vt")
                            nc.tensor.transpose(
                                vt[:],
                                vfm[:, hl * T + b * SEQ + 128 * jj:
                                    hl * T + b * SEQ + 128 * jj + 128],
                                ident[0:64, :])
                            # slot 2jj rows 0:64, slot 2jj+1 rows 64:128
                            (nc.vector.tensor_copy if jj % 2 else nc.scalar.copy)(
                                vaug[0:64, p, 65 * 2 * jj:65 * 2 * jj + 64],
                                vt[0:64, :])
                            (nc.scalar.copy if jj % 2 else nc.vector.tensor_copy)(
                                vaug[64:128, p,
                                     65 * (2 * jj + 1):65 * (2 * jj + 1) + 64],
                                vt[64:128, :])

            # ---- phase 3: attention per (batch, head) ----------------------
            with tc.tile_pool(name="sc_ps", bufs=2, space="PSUM") as scp, \
                    tc.tile_pool(name="ctx_ps", bufs=1, space="PSUM") as ctxp, \
                    tc.tile_pool(name="e_pool", bufs=3) as ep, \
                    tc.tile_pool(name="fin_pool", bufs=2) as fp:
                for b in range(BATCH):
                    for hl in range(HPC):
                        p = b * HPC + hl
                        qtok0 = hl * T + b * SEQ  # column base in q/k (head-major)
                        ctok0 = b * SEQ           # column base in ctx_all
                        ctx = ctxp.tile([65, SEQ], dt.float32, tag="ctx")
                        # PSUM start=True zeroes the whole 2KB bank: issue it
                        # exactly once per ctx bank (j=0 covers every column,
                        # so all banks start during the j=0 chunks).
                        ctx_bank_started = [False] * (SEQ // PSUM_BANK)
                        for ch in chunks:
                            j, W = ch["j"], ch["W"]
                            S = scp.tile([128, CHUNK_W], dt.float32, tag="S")
                            E = ep.tile([128, CHUNK_W], dt.bfloat16, tag="E")
                            kcol0 = qtok0 + 128 * j

                            def k_lhsT(side):
                                if side == 2:
                                    return k_sb[:, kcol0:kcol0 + 128]
                                return k_sb[:, kcol0 + 64 * side:
                                            kcol0 + 64 * side + 64]

                            def s_rows(side):
                                return (slice(0, 128) if side == 2
                                        else slice(64 * side, 64 * side + 64))

                            # scores (+fillers), split at psum banks
                            for side, r0, nblk, off in ch["segs"]:
                                for o, w in _bank_split(off, 64 * nblk):
                                    qc = qtok0 + 64 * r0 + (o - off)
                                    nc.tensor.matmul(
                                        S[s_rows(side), o:o + w],
                                        k_lhsT(side),
                                        q_sb[:, qc:qc + w],
                                        start=True, stop=True)
                            for side, off, w in ch["fillers"]:
                                for o, ww in _bank_split(off, w):
                                    nc.tensor.matmul(
                                        S[s_rows(side), o:o + ww],
                                        k_lhsT(side),
                                        q_sb[:, qtok0:qtok0 + ww],
                                        start=True, stop=True)
                            # exp
                            nc.scalar.activation(
                                E[:, :W], S[:, :W],
                                mybir.ActivationFunctionType.Exp, scale=SCALE)
                            # AV accumulate (+Z via ones column): K=128 with
                            # zero-padded v slots, everything base partition 0
                            for side, r0, nblk, off in ch["segs"]:
                                slots = ([2 * j, 2 * j + 1] if side == 2
                                         else [2 * j + side])
                                for m in slots:
                                    lhsT = vaug[:, p, 65 * m:65 * m + 65]
                                    for o, w in _bank_split(64 * r0,
                                                            64 * nblk):
                                        eo = off + (o - 64 * r0)
                                        bank = o // PSUM_BANK
                                        st = not ctx_bank_started[bank]
                                        ctx_bank_started[bank] = True
                                        nc.tensor.matmul(
                                            ctx[:, o:o + w], lhsT,
                                            E[:, eo:eo + w],
                                            start=st, stop=False,
                                            skip_group_check=True)
                        # finalize: 1/Z, broadcast multiply into ctx_all
                        rrow = fp.tile([1, SEQ], dt.float32, tag="rrow")
                        nc.vector.reciprocal(rrow[:], ctx[64:65, :])
                        rbc = fp.tile([64, SEQ], dt.float32, tag="rbc")
                        nc.gpsimd.partition_broadcast(rbc[:], rrow[:])
                        for cc in range(SEQ // 512):
                            csl = slice(512 * cc, 512 * (cc + 1))
                            nc.vector.tensor_tensor(
                                out=ctx_all[64 * hl:64 * hl + 64,
                                            ctok0 + 512 * cc:
                                            ctok0 + 512 * cc + 512],
                                in0=ctx[0:64, csl],
                                in1=rbc[:, csl],
                                op=mybir.AluOpType.mult)

            # ---- phase 4: out projection (partial, feature slice) ----------
            with tc.tile_pool(name="op_ps", bufs=2, space="PSUM") as opp, \
                    tc.tile_pool(name="op_sb", bufs=3) as opsb:
                for n in range(NCHUNK):
                    tsl = slice(512 * n, 512 * (n + 1))
                    for eo in range(8):
                        ps = opp.tile([128, 512], dt.float32, tag="op")
                        nc.tensor.matmul(
                            ps[:], woT[:, 128 * eo:128 * eo + 128],
                            ctx_all[:, tsl], start=True, stop=True)
                        ob = opsb.tile([128, 512], dt.bfloat16, tag="ob")
                        (nc.scalar.copy if eo % 2 else nc.vector.tensor_copy)(
                            ob[:], ps[:])
                        nc.sync.dma_start(
                            out=out_d[128 * eo:128 * eo + 128, tsl], in_=ob[:])

    nc.compile()
    return nc


_NC_CACHE = None


def make_in_maps(hs, wq, wk, wv, wo):
    hT = np.ascontiguousarray(
        np.asarray(hs, np.float32).reshape(T, EMBED_DIM).T).astype(BF16)
    ident = np.tile(np.eye(64, dtype=np.float32), (2, 1)).astype(BF16)
    wq = np.asarray(wq, np.float32)
    wk = np.asarray(wk, np.float32)
    wv = np.asarray(wv, np.float32)
    wo = np.asarray(wo, np.float32)
    in_maps = []
    for c in range(N_CORES):
        f = slice(FPC * c, FPC * (c + 1))
        wqkT = np.concatenate([wq[f, :].T, wk[f, :].T], axis=1)  # (1024, 256)
        in_maps.append({
            "hT": hT,
            "wqkT": np.ascontiguousarray(wqkT).astype(BF16),
            "wvT": np.ascontiguousarray(wv[f, :].T).astype(BF16),
            "woT": np.ascontiguousarray(wo[:, f].T).astype(BF16),
            "ident": ident,
        })
    return in_maps


def kernel(hidden_states, wq, bq, wk, bk, wv, bv, wo, bo):
    global _NC_CACHE
    hs = np.asarray(hidden_states, np.float32)
    wq = np.asarray(wq, np.float32)
    wk = np.asarray(wk, np.float32)
    wv = np.asarray(wv, np.float32)
    wo = np.asarray(wo, np.float32)
    bq = np.asarray(bq, np.float32)
    bk = np.asarray(bk, np.float32)
    bv = np.asarray(bv, np.float32)
    bo = np.asarray(bo, np.float32)
    assert hs.shape == (BATCH, SEQ, EMBED_DIM)
    # biases bq/bk/bv are zero in this problem; fold nonzero ones on host
    # by shifting is impossible (they pass through nonlinearities), so
    # guard loudly rather than silently returning wrong results.
    for name, bias in (("bq", bq), ("bk", bk), ("bv", bv)):
        if np.abs(bias).max() != 0:
            raise NotImplementedError(f"nonzero {name} not supported")

    from concourse.bass_utils import run_bass_kernel_spmd

    if _NC_CACHE is None:
        _NC_CACHE = _trace_core_program()
    nc = _NC_CACHE

    in_maps = make_in_maps(hs, wq, wk, wv, wo)
    res = run_bass_kernel_spmd(nc, in_maps, list(range(N_CORES)))
    acc = np.zeros((EMBED_DIM, T), np.float32)
    for c in range(N_CORES):
        acc += res.results[c]["out"].astype(np.float32)
    out = acc.T + bo[None, :]
    return out.reshape(BATCH, SEQ, EMBED_DIM).astype(np.float32)


# revision 25
# speedup vs baseline: 1.3365x; 1.3365x over previous
"""BigBird block-sparse attention for Trainium2, 8-core SPMD.

Sharding: head-parallel. Each core owns 2 of the 16 heads (both batches).
  - q/k/v projections computed only for the core's 128 feature slice
    (full hidden_states replicated, weights sliced column-wise).
  - attention fully local per (batch, head).
  - out_proj tensor-parallel on the head (contraction) dim: each core
    emits a full-shape partial; the host sums the 8 partials (cheaper
    than a 16MB on-device all-reduce) and adds the output bias.

On-device layout choices:
  - activations feature-major (features on partitions, tokens on free dim)
  - scores computed transposed: S_T[key, query] = k_j^T q, so that
    * AV is a natural matmul (contraction = keys = partitions),
    * the softmax denominator Z falls out of a ones-column appended to V^T,
    * normalization folds into the PSUM->SBUF context copy as a
      partition-broadcast multiply by 1/Z.
  - softmax skips max-subtraction (scores are O(1) after the 1/8 scale;
    exp cannot overflow fp32 for this distribution; softmax is shift
    invariant so the reference is matched).
  - BigBird mask is data independent and block-constant (64x64): it is
    evaluated at trace time into run-lists of attending query blocks per
    128-wide key tile.  No mask tensors on device at all.
"""

import numpy as np
import ml_dtypes
from contextlib import ExitStack

# ----- problem constants (hardcoded per contract) --------------------------
EMBED_DIM = 1024
NUM_HEADS = 16
HEAD_DIM = 64           # d per head
WINDOW = 3
N_RAND = 3
BLOCK = 64
BATCH = 2
SEQ = 2048
NB = SEQ // BLOCK       # 32 key/query blocks per sequence
N_CORES = 8
HPC = NUM_HEADS // N_CORES      # heads per core = 2
FPC = HPC * HEAD_DIM            # feature slice per core = 128
T = BATCH * SEQ                 # 4096 tokens
NKT = NB // 2                   # 16 key tiles of 128 keys per (b,h)
SCALE = HEAD_DIM ** -0.5

BF16 = ml_dtypes.bfloat16

# score-chunk window width in psum columns (2 PSUM banks)
CHUNK_W = 1024
PSUM_BANK = 512  # fp32 elements per bank


def _block_attend() -> np.ndarray:
    """attend[r, kb]: query block r attends key block kb.

    Block-granular replica of the reference _bigbird_mask (the mask is
    block-constant: global first block rows/cols, +-WINDOW band, and
    N_RAND random blocks per row drawn with RandomState(0))."""
    att = np.zeros((NB, NB), dtype=bool)
    att[0, :] = True
    att[:, 0] = True
    blk = np.arange(NB)
    att |= np.abs(blk[:, None] - blk[None, :]) <= WINDOW
    rng = np.random.RandomState(0)
    for b in range(1, NB):
        avail = [x for x in range(1, NB) if abs(x - b) > WINDOW]
        if avail:
            sel = rng.choice(avail, size=min(N_RAND, len(avail)), replace=False)
            att[b, sel] = True
    return att


def _runs_of(mask_1d: np.ndarray):
    """[(r0, nblocks)] maximal runs of consecutive True entries."""
    runs = []
    for r in np.flatnonzero(mask_1d):
        if runs and runs[-1][0] + runs[-1][1] == r:
            runs[-1][1] += 1
        else:
            runs.append([int(r), 1])
    return [(r0, n) for r0, n in runs]


def build_schedule():
    """Per key-tile j, pack score pieces into <=CHUNK_W-wide psum windows.

    Returns list of chunks; each chunk is a dict:
      j      : key tile index (keys j*128 .. j*128+128)
      W      : used width in psum columns
      segs   : [(side, r0, nblk, off)]  real score/AV pieces
               side: 2=full tile (128 keys), 0=low half (kb 2j), 1=high half
               r0   : first query block, nblk consecutive blocks
               off  : chunk-local psum column offset (64*... aligned)
      fillers: [(side, off, w)] dummy score MMs so exp never reads
               unwritten psum (output discarded; AV never touches them)
    """
    att = _block_attend()
    chunks = []
    for j in range(NKT):
        kb0, kb1 = 2 * j, 2 * j + 1
        a0, a1 = att[:, kb0], att[:, kb1]
        segs_all = []   # (side, r0, nblk) in absolute layout order
        # full pieces first, then the two half-piece streams sharing columns
        full_runs = _runs_of(a0 & a1)
        h0_runs = _runs_of(a0 & ~a1)
        h1_runs = _runs_of(a1 & ~a0)

        # absolute column layout: full region, then overlap region for halves
        layout = []  # (side, r0, nblk, abs_off)
        off = 0
        for r0, n in full_runs:
            layout.append((2, r0, n, off))
            off += 64 * n
        half_base = off
        off0 = off1 = half_base
        for r0, n in h0_runs:
            layout.append((0, r0, n, off0))
            off0 += 64 * n
        for r0, n in h1_runs:
            layout.append((1, r0, n, off1))
            off1 += 64 * n
        W_total = max(off0, off1)

        # slice the absolute layout into CHUNK_W windows
        n_windows = max(1, -(-W_total // CHUNK_W))
        for w_i in range(n_windows):
            lo, hi = w_i * CHUNK_W, min((w_i + 1) * CHUNK_W, W_total)
            segs = []
            cover = [np.zeros(hi - lo, bool), np.zeros(hi - lo, bool)]
            for side, r0, n, aoff in layout:
                s, e = aoff, aoff + 64 * n
                cs, ce = max(s, lo), min(e, hi)
                if cs >= ce:
                    continue
                # clip to window; r advances with columns (64 per block)
                r_lo = r0 + (cs - s) // 64
                nblk = (ce - cs) // 64
                segs.append((side, r_lo, nblk, cs - lo))
                for sd in ((0, 1) if side == 2 else (side,)):
                    cover[sd][cs - lo:ce - lo] = True
            if not segs:
                continue
            used = max(o + 64 * n for (_s, _r, n, o) in segs)
            fillers = []
            for sd in (0, 1):
                m = ~cover[sd][:used]
                i = 0
                while i < used:
                    if m[i]:
                        k = i
                        while k < used and m[k]:
                            k += 1
                        fillers.append((sd, i, k - i))
                        i = k
                    else:
                        i += 1
            chunks.append(dict(j=j, W=used, segs=segs, fillers=fillers))
    return chunks


def _bank_split(off, w, bank=PSUM_BANK):
    """split [off, off+w) at bank boundaries -> [(off, w), ...]"""
    out = []
    while w > 0:
        room = bank - (off % bank)
        take = min(room, w)
        out.append((off, take))
        off += take
        w -= take
    return out


# ---------------------------------------------------------------------------
# numpy golden of the exact on-device algorithm (fp32, validates schedule)
# ---------------------------------------------------------------------------
def numpy_golden(hidden_states, wq, bq, wk, bk, wv, bv, wo, bo):
    hs = np.asarray(hidden_states, np.float32).reshape(T, EMBED_DIM)
    chunks = build_schedule()
    out = np.zeros((T, EMBED_DIM), np.float32)
    for c in range(N_CORES):
        f = slice(FPC * c, FPC * (c + 1))
        q = hs @ np.asarray(wq, np.float32)[f, :].T  # (T, 128)
        k = hs @ np.asarray(wk, np.float32)[f, :].T
        v = hs @ np.asarray(wv, np.float32)[f, :].T
        partial = np.zeros((EMBED_DIM, T), np.float32)
        ctx_all = np.zeros((FPC, T), np.float32)
        for b in range(BATCH):
            for hl in range(HPC):
                d = slice(64 * hl, 64 * hl + 64)
                tok = slice(b * SEQ, (b + 1) * SEQ)
                qb = q[tok, d]   # (2048, 64)
                kb = k[tok, d]
                vb = v[tok, d]
                v_aug = np.concatenate([vb, np.ones((SEQ, 1), np.float32)], 1)
                ctx = np.zeros((65, SEQ), np.float32)
                for ch in chunks:
                    j = ch["j"]
                    E = np.zeros((128, ch["W"]), np.float32)
                    for side, r0, nblk, off in ch["segs"]:
                        kk = (slice(j * 128, j * 128 + 128) if side == 2 else
                              slice(j * 128 + 64 * side, j * 128 + 64 * side + 64))
                        qq = slice(64 * r0, 64 * (r0 + nblk))
                        s = kb[kk, :] @ qb[qq, :].T  # (keys, queries)
                        E[0 if side in (0, 2) else 64:][:s.shape[0], off:off + 64 * nblk] = \
                            np.exp(SCALE * s)
                    for side, r0, nblk, off in ch["segs"]:
                        kk = (slice(j * 128, j * 128 + 128) if side == 2 else
                              slice(j * 128 + 64 * side, j * 128 + 64 * side + 64))
                        rows = slice(0, 128) if side == 2 else \
                            slice(64 * side, 64 * side + 64)
                        qq = slice(64 * r0, 64 * (r0 + nblk))
                        ctx[:, qq] += v_aug[kk, :].T @ E[rows, off:off + 64 * nblk]
                ctx_n = ctx[:64, :] / ctx[64:65, :]
                ctx_all[d, tok] = ctx_n
        partial = np.asarray(wo, np.float32)[:, f] @ ctx_all  # (1024, T)
        out += partial.T
    out = out + np.asarray(bo, np.float32)
    return out.reshape(BATCH, SEQ, EMBED_DIM)


# ---------------------------------------------------------------------------
# Bass/Tile kernel (one core's program; SPMD across 8 cores)
# ---------------------------------------------------------------------------
def _trace_core_program():
    import concourse.bass as bass
    import concourse.mybir as mybir
    import concourse.tile as tile
    from concourse import bacc

    dt = mybir.dt
    chunks = build_schedule()

    nc = bacc.Bacc(None, target_bir_lowering=False)
    with tile.TileContext(nc) as tc:
        with ExitStack() as top:
            dram = top.enter_context(tc.tile_pool(name="dram", bufs=1, space="DRAM"))
            hT_d = dram.tile([EMBED_DIM, T], dt.bfloat16, kind="ExternalInput",
                             name="hT", uniquify=False)
            wqkT_d = dram.tile([EMBED_DIM, 2 * FPC], dt.bfloat16,
                               kind="ExternalInput", name="wqkT", uniquify=False)
            wvT_d = dram.tile([EMBED_DIM, FPC], dt.bfloat16,
                              kind="ExternalInput", name="wvT", uniquify=False)
            woT_d = dram.tile([FPC, EMBED_DIM], dt.bfloat16,
                              kind="ExternalInput", name="woT", uniquify=False)
            ident_d = dram.tile([128, 64], dt.bfloat16,
                                kind="ExternalInput", name="ident",
                                uniquify=False)
            out_d = dram.tile([EMBED_DIM, T], dt.bfloat16,
                              kind="ExternalOutput", name="out", uniquify=False)

            # ---- persistent SBUF tensors -----------------------------------
            persist = top.enter_context(tc.tile_pool(name="persist", bufs=1))
            wqk = persist.tile([128, 8, 2 * FPC], dt.bfloat16, name="wqk_sb")
            wvT = persist.tile([128, 8, FPC], dt.bfloat16, name="wv_sb")
            woT = persist.tile([128, EMBED_DIM], dt.bfloat16, name="wo_sb")
            # q/k/v head-major on 64 partitions (base-0 only: matmuls with
            # base-partition-64 contraction operands hit a codegen/HW bug)
            q_sb = persist.tile([64, HPC * T], dt.bfloat16, name="q_sb")
            k_sb = persist.tile([64, HPC * T], dt.bfloat16, name="k_sb")
            vfm = persist.tile([64, HPC * T], dt.bfloat16, name="vfm_sb")
            # per (b, hl): zero-padded v^T slots, one per 64-key block m:
            # rows (m%2)*64..+64 hold [v | 1], the other 64 rows are zero,
            # so every AV matmul is K=128 at base partition 0.
            vaug = persist.tile([128, BATCH * HPC, NB * 65], dt.bfloat16,
                                name="vaug_sb")
            ctx_all = persist.tile([128, T], dt.bfloat16, name="ctx_sb")
            ident = persist.tile([128, 64], dt.bfloat16, name="ident_sb")

            # input DMAs
            nc.sync.dma_start(out=wqk[:], in_=wqkT_d.rearrange(
                "(e p) f -> p e f", p=128))
            nc.sync.dma_start(out=wvT[:], in_=wvT_d.rearrange(
                "(e p) f -> p e f", p=128))
            nc.sync.dma_start(out=woT[:], in_=woT_d[:])
            nc.sync.dma_start(out=ident[:], in_=ident_d[:])

            # ---- phase 1: q/k/v projections (feature-major) ----------------
            NCHUNK = T // 512
            hT_pool = tc.tile_pool(name="hT_pool", bufs=1)
            with hT_pool as hp, \
                    tc.tile_pool(name="proj_ps", bufs=2, space="PSUM") as pps:
                hT = hp.tile([128, 8, T], dt.bfloat16, name="hT_sb")
                # chunked hT DMAs in (chunk, e) order so proj chunk 0 can
                # start after ~1/8 of the data instead of after all 8MB
                for n in range(NCHUNK):
                    for e in range(8):
                        nc.sync.dma_start(
                            out=hT[:, e, 512 * n:512 * n + 512],
                            in_=hT_d[128 * e:128 * e + 128,
                                     512 * n:512 * n + 512])
                for n in range(NCHUNK):
                    tsl = slice(512 * n, 512 * (n + 1))
                    for which, (wsl, dst) in enumerate(
                            [(slice(0, 128), q_sb), (slice(128, 256), k_sb),
                             (None, vfm)]):
                        ps = pps.tile([128, 512], dt.float32, tag="proj")
                        for e in range(8):
                            lhsT = (wqk[:, e, wsl] if wsl is not None
                                    else wvT[:, e, :])
                            nc.tensor.matmul(ps[:], lhsT, hT[:, e, tsl],
                                             start=(e == 0), stop=(e == 7))
                        # copy+cast psum -> sbuf per head half, alternating
                        # engines (dst is head-major on 64 partitions)
                        for hl in range(HPC):
                            eng = (2 * which + hl + n) % 2
                            src = ps[64 * hl:64 * hl + 64, :]
                            d2 = dst[:, hl * T + 512 * n: hl * T + 512 * n + 512]
                            if eng:
                                nc.scalar.copy(d2, src)
                            else:
                                nc.vector.tensor_copy(d2, src)

            # ---- phase 2: v -> token-major, zero-padded per-block slots ----
            with tc.tile_pool(name="vt_ps", bufs=4, space="PSUM") as vtp:
                # zero the pad halves + set the ones columns (disjoint from
                # the data ranges the copies below write)
                for p in range(BATCH * HPC):
                    slots = vaug[:, p, :].rearrange("p (m c) -> p m c", c=65)
                    nc.vector.memset(slots[0:64, 1::2, :], 0.0)
                    nc.vector.memset(slots[64:128, 0::2, :], 0.0)
                    nc.vector.memset(slots[0:64, 0::2, 64:65], 1.0)
                    nc.vector.memset(slots[64:128, 1::2, 64:65], 1.0)
                for b in range(BATCH):
                    for hl in range(HPC):
                        p = b * HPC + hl
                        for jj in range(NKT):
                            vt = vtp.tile([128, 64], dt.bfloat16, tag="vt")
                            nc.tensor.transpose(
                                vt[:],
                                vfm[:, hl * T + b * SEQ + 128 * jj:
                                    hl * T + b * SEQ + 128 * jj + 128],
                                ident[0:64, :])
                            # slot 2jj rows 0:64, slot 2jj+1 rows 64:128
                            (nc.vector.tensor_copy if jj % 2 else nc.scalar.copy)(
                                vaug[0:64, p, 65 * 2 * jj:65 * 2 * jj + 64],
                                vt[0:64, :])
                            (nc.scalar.copy if jj % 2 else nc.vector.tensor_copy)(
                                vaug[64:128, p,
                                     65 * (2 * jj + 1):65 * (2 * jj + 1) + 64],
                                vt[64:128, :])

            # ---- phase 3: attention per (batch, head) ----------------------
            with tc.tile_pool(name="sc_ps", bufs=2, space="PSUM") as scp, \
                    tc.tile_pool(name="ctx_ps", bufs=1, space="PSUM") as ctxp, \
                    tc.tile_pool(name="e_pool", bufs=1) as ep, \
                    tc.tile_pool(name="fin_pool", bufs=2) as fp:
                for b in range(BATCH):
                    for hl in range(HPC):
                        p = b * HPC + hl
                        qtok0 = hl * T + b * SEQ  # column base in q/k (head-major)
                        ctok0 = b * SEQ           # column base in ctx_all
                        ctx = ctxp.tile([65, SEQ], dt.float32, tag="ctx")
                        # PSUM start=True zeroes the whole 2KB bank: issue it
                        # exactly once per ctx bank (j=0 covers every column,
                        # so all banks start during the j=0 chunks).
                        ctx_bank_started = [False] * (SEQ // PSUM_BANK)

                        # ALL scores+exp first, then ALL AV: the AV block of
                        # this pair waits on the previous pair's finalize (ctx
                        # psum reuse), and PE is in-order — front-loading
                        # ~25us of score matmuls hides that chain.
                        E_tiles = []
                        for ci, ch in enumerate(chunks):
                            j, W = ch["j"], ch["W"]
                            S = scp.tile([128, CHUNK_W], dt.float32, tag="S")
                            E = ep.tile([128, W], dt.bfloat16,
                                        tag=f"E{ci}", name=f"E{ci}")
                            E_tiles.append(E)
                            kcol0 = qtok0 + 128 * j

                            def k_lhsT(side):
                                if side == 2:
                                    return k_sb[:, kcol0:kcol0 + 128]
                                return k_sb[:, kcol0 + 64 * side:
                                            kcol0 + 64 * side + 64]

                            def s_rows(side):
                                return (slice(0, 128) if side == 2
                                        else slice(64 * side, 64 * side + 64))

                            # scores (+fillers), split at psum banks
                            for side, r0, nblk, off in ch["segs"]:
                                for o, w in _bank_split(off, 64 * nblk):
                                    qc = qtok0 + 64 * r0 + (o - off)
                                    nc.tensor.matmul(
                                        S[s_rows(side), o:o + w],
                                        k_lhsT(side),
                                        q_sb[:, qc:qc + w],
                                        start=True, stop=True)
                            for side, off, w in ch["fillers"]:
                                for o, ww in _bank_split(off, w):
                                    nc.tensor.matmul(
                                        S[s_rows(side), o:o + ww],
                                        k_lhsT(side),
                                        q_sb[:, qtok0:qtok0 + ww],
                                        start=True, stop=True)
                            # exp
                            nc.scalar.activation(
                                E[:, :W], S[:, :W],
                                mybir.ActivationFunctionType.Exp, scale=SCALE)
                        # AV accumulate (+Z via ones column): K=128 with
                        # zero-padded v slots, everything base partition 0
                        for ci, ch in enumerate(chunks):
                            j = ch["j"]
                            E = E_tiles[ci]
                            for side, r0, nblk, off in ch["segs"]:
                                slots = ([2 * j, 2 * j + 1] if side == 2
                                         else [2 * j + side])
                                for m in slots:
                                    lhsT = vaug[:, p, 65 * m:65 * m + 65]
                                    for o, w in _bank_split(64 * r0,
                                                            64 * nblk):
                                        eo = off + (o - 64 * r0)
                                        bank = o // PSUM_BANK
                                        st = not ctx_bank_started[bank]
                                        ctx_bank_started[bank] = True
                                        nc.tensor.matmul(
                                            ctx[:, o:o + w], lhsT,
                                            E[:, eo:eo + w],
                                            start=st, stop=False,
                                            skip_group_check=True)
                        # finalize: 1/Z (spread over 64 partitions via DMA
                        # reshape: a (1,2048) reciprocal is single-lane and
                        # costs ~13us), then broadcast-multiply into ctx_all
                        zrow = fp.tile([1, SEQ], dt.float32, tag="zrow")
                        nc.vector.tensor_copy(zrow[:], ctx[64:65, :])
                        zsp = fp.tile([64, SEQ // 64], dt.float32, tag="zsp")
                        nc.sync.dma_start(out=zsp[:], in_=zrow[:])
                        rsp = fp.tile([64, SEQ // 64], dt.float32, tag="rsp")
                        nc.vector.reciprocal(rsp[:], zsp[:])
                        rrow = fp.tile([1, SEQ], dt.float32, tag="rrow")
                        nc.sync.dma_start(out=rrow[:], in_=rsp[:])
                        rbc = fp.tile([64, SEQ], dt.float32, tag="rbc")
                        nc.gpsimd.partition_broadcast(rbc[:], rrow[:])
                        for cc in range(SEQ // 512):
                            csl = slice(512 * cc, 512 * (cc + 1))
                            nc.vector.tensor_tensor(
                                out=ctx_all[64 * hl:64 * hl + 64,
                                            ctok0 + 512 * cc:
                                            ctok0 + 512 * cc + 512],
                                in0=ctx[0:64, csl],
                                in1=rbc[:, csl],
                                op=mybir.AluOpType.mult)

            # ---- phase 4: out projection (partial, feature slice) ----------
            with tc.tile_pool(name="op_ps", bufs=4, space="PSUM") as opp, \
                    tc.tile_pool(name="op_sb", bufs=4) as opsb:
                for n in range(NCHUNK):
                    tsl = slice(512 * n, 512 * (n + 1))
                    for eo in range(8):
                        ps = opp.tile([128, 512], dt.float32, tag="op")
                        nc.tensor.matmul(
                            ps[:], woT[:, 128 * eo:128 * eo + 128],
                            ctx_all[:, tsl], start=True, stop=True)
                        ob = opsb.tile([128, 512], dt.bfloat16, tag="ob")
                        (nc.scalar.copy if eo % 2 else nc.vector.tensor_copy)(
                            ob[:], ps[:])
                        nc.sync.dma_start(
                            out=out_d[128 * eo:128 * eo + 128, tsl], in_=ob[:])

    nc.compile()
    return nc


_NC_CACHE = None


def make_in_maps(hs, wq, wk, wv, wo):
    hT = np.ascontiguousarray(
        np.asarray(hs, np.float32).reshape(T, EMBED_DIM).T).astype(BF16)
    ident = np.tile(np.eye(64, dtype=np.float32), (2, 1)).astype(BF16)
    wq = np.asarray(wq, np.float32)
    wk = np.asarray(wk, np.float32)
    wv = np.asarray(wv, np.float32)
    wo = np.asarray(wo, np.float32)
    in_maps = []
    for c in range(N_CORES):
        f = slice(FPC * c, FPC * (c + 1))
        wqkT = np.concatenate([wq[f, :].T, wk[f, :].T], axis=1)  # (1024, 256)
        in_maps.append({
            "hT": hT,
            "wqkT": np.ascontiguousarray(wqkT).astype(BF16),
            "wvT": np.ascontiguousarray(wv[f, :].T).astype(BF16),
            "woT": np.ascontiguousarray(wo[:, f].T).astype(BF16),
            "ident": ident,
        })
    return in_maps


def kernel(hidden_states, wq, bq, wk, bk, wv, bv, wo, bo):
    global _NC_CACHE
    hs = np.asarray(hidden_states, np.float32)
    wq = np.asarray(wq, np.float32)
    wk = np.asarray(wk, np.float32)
    wv = np.asarray(wv, np.float32)
    wo = np.asarray(wo, np.float32)
    bq = np.asarray(bq, np.float32)
    bk = np.asarray(bk, np.float32)
    bv = np.asarray(bv, np.float32)
    bo = np.asarray(bo, np.float32)
    assert hs.shape == (BATCH, SEQ, EMBED_DIM)
    # biases bq/bk/bv are zero in this problem; fold nonzero ones on host
    # by shifting is impossible (they pass through nonlinearities), so
    # guard loudly rather than silently returning wrong results.
    for name, bias in (("bq", bq), ("bk", bk), ("bv", bv)):
        if np.abs(bias).max() != 0:
            raise NotImplementedError(f"nonzero {name} not supported")

    from concourse.bass_utils import run_bass_kernel_spmd

    if _NC_CACHE is None:
        _NC_CACHE = _trace_core_program()
    nc = _NC_CACHE

    in_maps = make_in_maps(hs, wq, wk, wv, wo)
    res = run_bass_kernel_spmd(nc, in_maps, list(range(N_CORES)))
    acc = np.zeros((EMBED_DIM, T), np.float32)
    for c in range(N_CORES):
        acc += res.results[c]["out"].astype(np.float32)
    out = acc.T + bo[None, :]
    return out.reshape(BATCH, SEQ, EMBED_DIM).astype(np.float32)


# revision 32
# speedup vs baseline: 1.3798x; 1.0324x over previous
"""BigBird block-sparse attention for Trainium2, 8-core SPMD.

Sharding: head-parallel. Each core owns 2 of the 16 heads (both batches).
  - q/k/v projections computed only for the core's 128 feature slice
    (full hidden_states replicated, weights sliced column-wise).
  - attention fully local per (batch, head).
  - out_proj tensor-parallel on the head (contraction) dim: each core
    emits a full-shape partial; the host sums the 8 partials (cheaper
    than a 16MB on-device all-reduce) and adds the output bias.

On-device layout choices:
  - activations feature-major (features on partitions, tokens on free dim)
  - scores computed transposed: S_T[key, query] = k_j^T q, so that
    * AV is a natural matmul (contraction = keys = partitions),
    * the softmax denominator Z falls out of a ones-column appended to V^T,
    * normalization folds into the PSUM->SBUF context copy as a
      partition-broadcast multiply by 1/Z.
  - softmax skips max-subtraction (scores are O(1) after the 1/8 scale;
    exp cannot overflow fp32 for this distribution; softmax is shift
    invariant so the reference is matched).
  - BigBird mask is data independent and block-constant (64x64): it is
    evaluated at trace time into run-lists of attending query blocks per
    128-wide key tile.  No mask tensors on device at all.
"""

import numpy as np
import ml_dtypes
from contextlib import ExitStack

# ----- problem constants (hardcoded per contract) --------------------------
EMBED_DIM = 1024
NUM_HEADS = 16
HEAD_DIM = 64           # d per head
WINDOW = 3
N_RAND = 3
BLOCK = 64
BATCH = 2
SEQ = 2048
NB = SEQ // BLOCK       # 32 key/query blocks per sequence
N_CORES = 8
HPC = NUM_HEADS // N_CORES      # heads per core = 2
FPC = HPC * HEAD_DIM            # feature slice per core = 128
T = BATCH * SEQ                 # 4096 tokens
NKT = NB // 2                   # 16 key tiles of 128 keys per (b,h)
SCALE = HEAD_DIM ** -0.5

BF16 = ml_dtypes.bfloat16

# score-chunk window width in psum columns (2 PSUM banks)
CHUNK_W = 1024
PSUM_BANK = 512  # fp32 elements per bank


def _block_attend() -> np.ndarray:
    """attend[r, kb]: query block r attends key block kb.

    Block-granular replica of the reference _bigbird_mask (the mask is
    block-constant: global first block rows/cols, +-WINDOW band, and
    N_RAND random blocks per row drawn with RandomState(0))."""
    att = np.zeros((NB, NB), dtype=bool)
    att[0, :] = True
    att[:, 0] = True
    blk = np.arange(NB)
    att |= np.abs(blk[:, None] - blk[None, :]) <= WINDOW
    rng = np.random.RandomState(0)
    for b in range(1, NB):
        avail = [x for x in range(1, NB) if abs(x - b) > WINDOW]
        if avail:
            sel = rng.choice(avail, size=min(N_RAND, len(avail)), replace=False)
            att[b, sel] = True
    return att


def _runs_of(mask_1d: np.ndarray):
    """[(r0, nblocks)] maximal runs of consecutive True entries."""
    runs = []
    for r in np.flatnonzero(mask_1d):
        if runs and runs[-1][0] + runs[-1][1] == r:
            runs[-1][1] += 1
        else:
            runs.append([int(r), 1])
    return [(r0, n) for r0, n in runs]


def build_schedule():
    """Per key-tile j, pack score pieces into <=CHUNK_W-wide psum windows.

    Returns list of chunks; each chunk is a dict:
      j      : key tile index (keys j*128 .. j*128+128)
      W      : used width in psum columns
      segs   : [(side, r0, nblk, off)]  real score/AV pieces
               side: 2=full tile (128 keys), 0=low half (kb 2j), 1=high half
               r0   : first query block, nblk consecutive blocks
               off  : chunk-local psum column offset (64*... aligned)
      fillers: [(side, off, w)] dummy score MMs so exp never reads
               unwritten psum (output discarded; AV never touches them)
    """
    att = _block_attend()
    chunks = []
    for j in range(NKT):
        kb0, kb1 = 2 * j, 2 * j + 1
        a0, a1 = att[:, kb0], att[:, kb1]
        segs_all = []   # (side, r0, nblk) in absolute layout order
        # full pieces first, then the two half-piece streams sharing columns
        full_runs = _runs_of(a0 & a1)
        h0_runs = _runs_of(a0 & ~a1)
        h1_runs = _runs_of(a1 & ~a0)

        # absolute column layout: full region, then overlap region for halves
        layout = []  # (side, r0, nblk, abs_off)
        off = 0
        for r0, n in full_runs:
            layout.append((2, r0, n, off))
            off += 64 * n
        half_base = off
        off0 = off1 = half_base
        for r0, n in h0_runs:
            layout.append((0, r0, n, off0))
            off0 += 64 * n
        for r0, n in h1_runs:
            layout.append((1, r0, n, off1))
            off1 += 64 * n
        W_total = max(off0, off1)

        # slice the absolute layout into CHUNK_W windows
        n_windows = max(1, -(-W_total // CHUNK_W))
        for w_i in range(n_windows):
            lo, hi = w_i * CHUNK_W, min((w_i + 1) * CHUNK_W, W_total)
            segs = []
            cover = [np.zeros(hi - lo, bool), np.zeros(hi - lo, bool)]
            for side, r0, n, aoff in layout:
                s, e = aoff, aoff + 64 * n
                cs, ce = max(s, lo), min(e, hi)
                if cs >= ce:
                    continue
                # clip to window; r advances with columns (64 per block)
                r_lo = r0 + (cs - s) // 64
                nblk = (ce - cs) // 64
                segs.append((side, r_lo, nblk, cs - lo))
                for sd in ((0, 1) if side == 2 else (side,)):
                    cover[sd][cs - lo:ce - lo] = True
            if not segs:
                continue
            used = max(o + 64 * n for (_s, _r, n, o) in segs)
            fillers = []
            for sd in (0, 1):
                m = ~cover[sd][:used]
                i = 0
                while i < used:
                    if m[i]:
                        k = i
                        while k < used and m[k]:
                            k += 1
                        fillers.append((sd, i, k - i))
                        i = k
                    else:
                        i += 1
            chunks.append(dict(j=j, W=used, segs=segs, fillers=fillers))
    return chunks


def _bank_split(off, w, bank=PSUM_BANK):
    """split [off, off+w) at bank boundaries -> [(off, w), ...]"""
    out = []
    while w > 0:
        room = bank - (off % bank)
        take = min(room, w)
        out.append((off, take))
        off += take
        w -= take
    return out


# ---------------------------------------------------------------------------
# numpy golden of the exact on-device algorithm (fp32, validates schedule)
# ---------------------------------------------------------------------------
def numpy_golden(hidden_states, wq, bq, wk, bk, wv, bv, wo, bo):
    hs = np.asarray(hidden_states, np.float32).reshape(T, EMBED_DIM)
    chunks = build_schedule()
    out = np.zeros((T, EMBED_DIM), np.float32)
    for c in range(N_CORES):
        f = slice(FPC * c, FPC * (c + 1))
        q = hs @ np.asarray(wq, np.float32)[f, :].T  # (T, 128)
        k = hs @ np.asarray(wk, np.float32)[f, :].T
        v = hs @ np.asarray(wv, np.float32)[f, :].T
        partial = np.zeros((EMBED_DIM, T), np.float32)
        ctx_all = np.zeros((FPC, T), np.float32)
        for b in range(BATCH):
            for hl in range(HPC):
                d = slice(64 * hl, 64 * hl + 64)
                tok = slice(b * SEQ, (b + 1) * SEQ)
                qb = q[tok, d]   # (2048, 64)
                kb = k[tok, d]
                vb = v[tok, d]
                v_aug = np.concatenate([vb, np.ones((SEQ, 1), np.float32)], 1)
                ctx = np.zeros((65, SEQ), np.float32)
                for ch in chunks:
                    j = ch["j"]
                    E = np.zeros((128, ch["W"]), np.float32)
                    for side, r0, nblk, off in ch["segs"]:
                        kk = (slice(j * 128, j * 128 + 128) if side == 2 else
                              slice(j * 128 + 64 * side, j * 128 + 64 * side + 64))
                        qq = slice(64 * r0, 64 * (r0 + nblk))
                        s = kb[kk, :] @ qb[qq, :].T  # (keys, queries)
                        E[0 if side in (0, 2) else 64:][:s.shape[0], off:off + 64 * nblk] = \
                            np.exp(SCALE * s)
                    for side, r0, nblk, off in ch["segs"]:
                        kk = (slice(j * 128, j * 128 + 128) if side == 2 else
                              slice(j * 128 + 64 * side, j * 128 + 64 * side + 64))
                        rows = slice(0, 128) if side == 2 else \
                            slice(64 * side, 64 * side + 64)
                        qq = slice(64 * r0, 64 * (r0 + nblk))
                        ctx[:, qq] += v_aug[kk, :].T @ E[rows, off:off + 64 * nblk]
                ctx_n = ctx[:64, :] / ctx[64:65, :]
                ctx_all[d, tok] = ctx_n
        partial = np.asarray(wo, np.float32)[:, f] @ ctx_all  # (1024, T)
        out += partial.T
    out = out + np.asarray(bo, np.float32)
    return out.reshape(BATCH, SEQ, EMBED_DIM)


# ---------------------------------------------------------------------------
# Bass/Tile kernel (one core's program; SPMD across 8 cores)
# ---------------------------------------------------------------------------
def _trace_core_program():
    import concourse.bass as bass
    import concourse.mybir as mybir
    import concourse.tile as tile
    from concourse import bacc

    dt = mybir.dt
    chunks = build_schedule()

    nc = bacc.Bacc(None, target_bir_lowering=False)
    with tile.TileContext(nc) as tc:
        with ExitStack() as top:
            dram = top.enter_context(tc.tile_pool(name="dram", bufs=1, space="DRAM"))
            hT_d = dram.tile([EMBED_DIM, T], dt.bfloat16, kind="ExternalInput",
                             name="hT", uniquify=False)
            wqkT_d = dram.tile([EMBED_DIM, 2 * FPC], dt.bfloat16,
                               kind="ExternalInput", name="wqkT", uniquify=False)
            wvT_d = dram.tile([EMBED_DIM, FPC], dt.bfloat16,
                              kind="ExternalInput", name="wvT", uniquify=False)
            woT_d = dram.tile([FPC, EMBED_DIM], dt.bfloat16,
                              kind="ExternalInput", name="woT", uniquify=False)
            ident_d = dram.tile([128, 64], dt.bfloat16,
                                kind="ExternalInput", name="ident",
                                uniquify=False)
            out_d = dram.tile([EMBED_DIM, T], dt.bfloat16,
                              kind="ExternalOutput", name="out", uniquify=False)

            # ---- persistent SBUF tensors -----------------------------------
            persist = top.enter_context(tc.tile_pool(name="persist", bufs=1))
            wqk = persist.tile([128, 8, 2 * FPC], dt.bfloat16, name="wqk_sb")
            wvT = persist.tile([128, 8, FPC], dt.bfloat16, name="wv_sb")
            woT = persist.tile([128, EMBED_DIM], dt.bfloat16, name="wo_sb")
            # q/k/v head-major on 64 partitions (base-0 only: matmuls with
            # base-partition-64 contraction operands hit a codegen/HW bug)
            q_sb = persist.tile([64, HPC * T], dt.bfloat16, name="q_sb")
            k_sb = persist.tile([64, HPC * T], dt.bfloat16, name="k_sb")
            vfm = persist.tile([64, HPC * T], dt.bfloat16, name="vfm_sb")
            # per (b, hl): zero-padded v^T slots, one per 64-key block m:
            # rows (m%2)*64..+64 hold [v | 1], the other 64 rows are zero,
            # so every AV matmul is K=128 at base partition 0.
            vaug = persist.tile([128, BATCH * HPC, NB * 65], dt.bfloat16,
                                name="vaug_sb")
            # interleaved (both blocks of a key tile) for full AV pieces:
            # one K=128 matmul + one 65-col LDWEIGHTS instead of two
            vaug2 = persist.tile([128, BATCH * HPC, NKT * 65], dt.bfloat16,
                                 name="vaug2_sb")
            ctx_all = persist.tile([128, T], dt.bfloat16, name="ctx_sb")
            ident = persist.tile([128, 64], dt.bfloat16, name="ident_sb")

            # input DMAs (wo/ident are not needed until later phases — put
            # them on the gpsimd queue so they don't delay hT/proj)
            nc.sync.dma_start(out=wqk[:], in_=wqkT_d.rearrange(
                "(e p) f -> p e f", p=128))
            nc.sync.dma_start(out=wvT[:], in_=wvT_d.rearrange(
                "(e p) f -> p e f", p=128))
            nc.gpsimd.dma_start(out=woT[:], in_=woT_d[:])
            nc.gpsimd.dma_start(out=ident[:], in_=ident_d[:])

            # ---- phase 1: q/k/v projections (feature-major) ----------------
            NCHUNK = T // 512
            hT_pool = tc.tile_pool(name="hT_pool", bufs=1)
            with hT_pool as hp, \
                    tc.tile_pool(name="proj_ps", bufs=2, space="PSUM") as pps:
                hT = hp.tile([128, 8, T], dt.bfloat16, name="hT_sb")
                # chunked hT DMAs in (chunk, e) order so proj chunk 0 can
                # start after ~1/8 of the data instead of after all 8MB
                for n in range(NCHUNK):
                    for e in range(8):
                        nc.sync.dma_start(
                            out=hT[:, e, 512 * n:512 * n + 512],
                            in_=hT_d[128 * e:128 * e + 128,
                                     512 * n:512 * n + 512])
                for n in range(NCHUNK):
                    tsl = slice(512 * n, 512 * (n + 1))
                    for which, (wsl, dst) in enumerate(
                            [(slice(0, 128), q_sb), (slice(128, 256), k_sb),
                             (None, vfm)]):
                        ps = pps.tile([128, 512], dt.float32, tag="proj")
                        for e in range(8):
                            lhsT = (wqk[:, e, wsl] if wsl is not None
                                    else wvT[:, e, :])
                            nc.tensor.matmul(ps[:], lhsT, hT[:, e, tsl],
                                             start=(e == 0), stop=(e == 7))
                        # copy+cast psum -> sbuf per head half, alternating
                        # engines (dst is head-major on 64 partitions)
                        for hl in range(HPC):
                            eng = (2 * which + hl + n) % 2
                            src = ps[64 * hl:64 * hl + 64, :]
                            d2 = dst[:, hl * T + 512 * n: hl * T + 512 * n + 512]
                            if eng:
                                nc.scalar.copy(d2, src)
                            else:
                                nc.vector.tensor_copy(d2, src)

            # ---- phase 2: v -> token-major, zero-padded per-block slots ----
            with tc.tile_pool(name="vt_ps", bufs=4, space="PSUM") as vtp:
                # zero the pad halves + set the ones columns (disjoint from
                # the data ranges the copies below write)
                for p in range(BATCH * HPC):
                    slots = vaug[:, p, :].rearrange("p (m c) -> p m c", c=65)
                    nc.vector.memset(slots[0:64, 1::2, :], 0.0)
                    nc.vector.memset(slots[64:128, 0::2, :], 0.0)
                    nc.vector.memset(slots[0:64, 0::2, 64:65], 1.0)
                    nc.vector.memset(slots[64:128, 1::2, 64:65], 1.0)
                    s2 = vaug2[:, p, :].rearrange("p (m c) -> p m c", c=65)
                    nc.vector.memset(s2[:, :, 64:65], 1.0)
                for b in range(BATCH):
                    for hl in range(HPC):
                        p = b * HPC + hl
                        for jj in range(NKT):
                            vt = vtp.tile([128, 64], dt.bfloat16, tag="vt")
                            nc.tensor.transpose(
                                vt[:],
                                vfm[:, hl * T + b * SEQ + 128 * jj:
                                    hl * T + b * SEQ + 128 * jj + 128],
                                ident[0:64, :])
                            # slot 2jj rows 0:64, slot 2jj+1 rows 64:128
                            (nc.vector.tensor_copy if jj % 2 else nc.scalar.copy)(
                                vaug[0:64, p, 65 * 2 * jj:65 * 2 * jj + 64],
                                vt[0:64, :])
                            (nc.scalar.copy if jj % 2 else nc.vector.tensor_copy)(
                                vaug[64:128, p,
                                     65 * (2 * jj + 1):65 * (2 * jj + 1) + 64],
                                vt[64:128, :])
                            (nc.vector.tensor_copy if jj % 2 else nc.scalar.copy)(
                                vaug2[:, p, 65 * jj:65 * jj + 64], vt[:])

            # ---- phase 3: attention per (batch, head) ----------------------
            with tc.tile_pool(name="sc_ps", bufs=2, space="PSUM") as scp, \
                    tc.tile_pool(name="ctx_ps", bufs=1, space="PSUM") as ctxp, \
                    tc.tile_pool(name="e_pool", bufs=1) as ep, \
                    tc.tile_pool(name="fin_pool", bufs=2) as fp:
                for b in range(BATCH):
                    for hl in range(HPC):
                        p = b * HPC + hl
                        qtok0 = hl * T + b * SEQ  # column base in q/k (head-major)
                        ctok0 = b * SEQ           # column base in ctx_all
                        ctx = ctxp.tile([65, SEQ], dt.float32, tag="ctx")
                        # PSUM start=True zeroes the whole 2KB bank: issue it
                        # exactly once per ctx bank (j=0 covers every column,
                        # so all banks start during the j=0 chunks).
                        ctx_bank_started = [False] * (SEQ // PSUM_BANK)

                        # ALL scores+exp first, then ALL AV: the AV block of
                        # this pair waits on the previous pair's finalize (ctx
                        # psum reuse), and PE is in-order — front-loading
                        # ~25us of score matmuls hides that chain.
                        E_tiles = []
                        for ci, ch in enumerate(chunks):
                            j, W = ch["j"], ch["W"]
                            S = scp.tile([128, CHUNK_W], dt.float32, tag="S")
                            E = ep.tile([128, W], dt.bfloat16,
                                        tag=f"E{ci}", name=f"E{ci}")
                            E_tiles.append(E)
                            kcol0 = qtok0 + 128 * j

                            def k_lhsT(side):
                                if side == 2:
                                    return k_sb[:, kcol0:kcol0 + 128]
                                return k_sb[:, kcol0 + 64 * side:
                                            kcol0 + 64 * side + 64]

                            def s_rows(side):
                                return (slice(0, 128) if side == 2
                                        else slice(64 * side, 64 * side + 64))

                            # scores (+fillers), split at psum banks
                            for side, r0, nblk, off in ch["segs"]:
                                for o, w in _bank_split(off, 64 * nblk):
                                    qc = qtok0 + 64 * r0 + (o - off)
                                    nc.tensor.matmul(
                                        S[s_rows(side), o:o + w],
                                        k_lhsT(side),
                                        q_sb[:, qc:qc + w],
                                        start=True, stop=True)
                            for side, off, w in ch["fillers"]:
                                for o, ww in _bank_split(off, w):
                                    nc.tensor.matmul(
                                        S[s_rows(side), o:o + ww],
                                        k_lhsT(side),
                                        q_sb[:, qtok0:qtok0 + ww],
                                        start=True, stop=True)
                            # exp
                            nc.scalar.activation(
                                E[:, :W], S[:, :W],
                                mybir.ActivationFunctionType.Exp, scale=SCALE)
                        # AV accumulate (+Z via ones column): K=128 with
                        # zero-padded v slots, everything base partition 0
                        for ci, ch in enumerate(chunks):
                            j = ch["j"]
                            E = E_tiles[ci]
                            for side, r0, nblk, off in ch["segs"]:
                                if side == 2:
                                    lhsTs = [vaug2[:, p, 65 * j:65 * j + 65]]
                                else:
                                    m = 2 * j + side
                                    lhsTs = [vaug[:, p, 65 * m:65 * m + 65]]
                                for lhsT in lhsTs:
                                    for o, w in _bank_split(64 * r0,
                                                            64 * nblk):
                                        eo = off + (o - 64 * r0)
                                        bank = o // PSUM_BANK
                                        st = not ctx_bank_started[bank]
                                        ctx_bank_started[bank] = True
                                        nc.tensor.matmul(
                                            ctx[:, o:o + w], lhsT,
                                            E[:, eo:eo + w],
                                            start=st, stop=False,
                                            skip_group_check=True)
                        # finalize: 1/Z (spread over 64 partitions via DMA
                        # reshape: a (1,2048) reciprocal is single-lane and
                        # costs ~13us), then broadcast-multiply into ctx_all
                        zrow = fp.tile([1, SEQ], dt.float32, tag="zrow")
                        nc.vector.tensor_copy(zrow[:], ctx[64:65, :])
                        zsp = fp.tile([64, SEQ // 64], dt.float32, tag="zsp")
                        nc.sync.dma_start(out=zsp[:], in_=zrow[:])
                        rsp = fp.tile([64, SEQ // 64], dt.float32, tag="rsp")
                        nc.vector.reciprocal(rsp[:], zsp[:])
                        rrow = fp.tile([1, SEQ], dt.float32, tag="rrow")
                        nc.sync.dma_start(out=rrow[:], in_=rsp[:])
                        rbc = fp.tile([64, SEQ], dt.float32, tag="rbc")
                        nc.gpsimd.partition_broadcast(rbc[:], rrow[:])
                        for cc in range(SEQ // 512):
                            csl = slice(512 * cc, 512 * (cc + 1))
                            nc.vector.tensor_tensor(
                                out=ctx_all[64 * hl:64 * hl + 64,
                                            ctok0 + 512 * cc:
                                            ctok0 + 512 * cc + 512],
                                in0=ctx[0:64, csl],
                                in1=rbc[:, csl],
                                op=mybir.AluOpType.mult)

            # ---- phase 4: out projection (partial, feature slice) ----------
            with tc.tile_pool(name="op_ps", bufs=6, space="PSUM") as opp, \
                    tc.tile_pool(name="op_sb", bufs=6) as opsb:
                for n in range(NCHUNK):
                    tsl = slice(512 * n, 512 * (n + 1))
                    for eo in range(8):
                        ps = opp.tile([128, 512], dt.float32, tag="op")
                        nc.tensor.matmul(
                            ps[:], woT[:, 128 * eo:128 * eo + 128],
                            ctx_all[:, tsl], start=True, stop=True)
                        ob = opsb.tile([128, 512], dt.bfloat16, tag="ob")
                        (nc.scalar.copy if eo % 2 else nc.vector.tensor_copy)(
                            ob[:], ps[:])
                        # spread DMA dispatch across sync and gpsimd queues
                        eng = nc.sync if eo % 2 else nc.gpsimd
                        eng.dma_start(
                            out=out_d[128 * eo:128 * eo + 128, tsl], in_=ob[:])

    nc.compile()
    return nc


_NC_CACHE = None


def make_in_maps(hs, wq, wk, wv, wo):
    hT = np.ascontiguousarray(
        np.asarray(hs, np.float32).reshape(T, EMBED_DIM).T).astype(BF16)
    ident = np.tile(np.eye(64, dtype=np.float32), (2, 1)).astype(BF16)
    wq = np.asarray(wq, np.float32)
    wk = np.asarray(wk, np.float32)
    wv = np.asarray(wv, np.float32)
    wo = np.asarray(wo, np.float32)
    in_maps = []
    for c in range(N_CORES):
        f = slice(FPC * c, FPC * (c + 1))
        wqkT = np.concatenate([wq[f, :].T, wk[f, :].T], axis=1)  # (1024, 256)
        in_maps.append({
            "hT": hT,
            "wqkT": np.ascontiguousarray(wqkT).astype(BF16),
            "wvT": np.ascontiguousarray(wv[f, :].T).astype(BF16),
            "woT": np.ascontiguousarray(wo[:, f].T).astype(BF16),
            "ident": ident,
        })
    return in_maps


def kernel(hidden_states, wq, bq, wk, bk, wv, bv, wo, bo):
    global _NC_CACHE
    hs = np.asarray(hidden_states, np.float32)
    wq = np.asarray(wq, np.float32)
    wk = np.asarray(wk, np.float32)
    wv = np.asarray(wv, np.float32)
    wo = np.asarray(wo, np.float32)
    bq = np.asarray(bq, np.float32)
    bk = np.asarray(bk, np.float32)
    bv = np.asarray(bv, np.float32)
    bo = np.asarray(bo, np.float32)
    assert hs.shape == (BATCH, SEQ, EMBED_DIM)
    # biases bq/bk/bv are zero in this problem; fold nonzero ones on host
    # by shifting is impossible (they pass through nonlinearities), so
    # guard loudly rather than silently returning wrong results.
    for name, bias in (("bq", bq), ("bk", bk), ("bv", bv)):
        if np.abs(bias).max() != 0:
            raise NotImplementedError(f"nonzero {name} not supported")

    from concourse.bass_utils import run_bass_kernel_spmd

    if _NC_CACHE is None:
        _NC_CACHE = _trace_core_program()
    nc = _NC_CACHE

    in_maps = make_in_maps(hs, wq, wk, wv, wo)
    res = run_bass_kernel_spmd(nc, in_maps, list(range(N_CORES)))
    acc = np.zeros((EMBED_DIM, T), np.float32)
    for c in range(N_CORES):
        acc += res.results[c]["out"].astype(np.float32)
    out = acc.T + bo[None, :]
    return out.reshape(BATCH, SEQ, EMBED_DIM).astype(np.float32)
